# revision 6
# baseline (speedup 1.0000x reference)
"""Causal self-attention Trainium2 kernel (8 NeuronCores), v2.

Sharding (Megatron-style): core c -> batch b = c//2, head-group g = c%2
(8 of 16 heads). W_q/W_k/W_v column-sliced per head group; W_o row-sliced;
host sums the two partial outputs per batch and adds b_o.

v2 changes over the bf16 baseline (351925 ns):
  * q/k projections run in fp8e4m3 with perf_mode=DoubleRow (two packed
    contraction tiles per pass, 2x PE throughput). W_q/W_k are prescaled
    x32 host-side so their ~U(-1/32,1/32) values land in e4m3's normal
    range; the 32*32 factor is folded into the softmax exp scale.
    Scores/v/ctx/out-proj stay bf16 (fp8 there either busts the 2e-2
    error budget or needs a base-partition-96 operand the ISA forbids).
    This config measures ~1e-2 in numpy emulation vs the 3.6e-3 baseline.
  * Software-pipelined emission: per score-group (head-pair, k-tile),
    scores(g+1) is emitted before ctx(g) so the PE never sits behind the
    exp; q/k/v projections of chunk qc+1 and out-proj of chunk qc-1 are
    interleaved as PE filler inside chunk qc's Act-bound attention.
  * exp APs are trimmed on diagonal k-tiles ([128, 2, 512-t0]).
  * fp16 partial outputs (halves the output DMA; partials summed f32 host
    side, b_o added there).

Per-core kernel layout:
  xt8   [512, 4096]  x^T fp8 pairs: row pair*128+p, col sub*2048+n
  wqk8  [512, 2048]  [Wq'|Wk'] fp8 pairs, x32, strip-permuted columns
  xtb   [1024, 2048] x^T bf16 (v projection)
  wv    [1024, 512]  W_v slice bf16
  wo    [512, 1024]  W_o rows bf16
  out   [2048, 1024] fp16 partial

`reps` repeats the body inside one NEFF for (T(3)-T(1))/2 timing; graded
path is reps=1.
"""

import sys

import numpy as np

sys.path.insert(0, "/opt/trn_rl_repo")

import ml_dtypes

BF16 = ml_dtypes.bfloat16
F8E4 = ml_dtypes.float8_e4m3

D_EMB = 1024
N_SEQ = 2048
HD = 64
NPAIR = 4  # d_emb DoubleRow pairs (256 each)
KT = 8  # d_emb 128-tiles (v projection)
NT = N_SEQ // 128  # 16 n-tiles
QC = N_SEQ // 512  # 4 query chunks
WS = 32.0  # host prescale on W_q/W_k before fp8 quantization
SCALE_EXP = 0.125 / (WS * WS)  # 1/sqrt(64) / (32*32)

_CACHE = {}


def _emit_body(nc, tc, mybir, sfx, rep, pools, tiles, xt8_d, wqk8_d, xtb_d, wv_d, wo_d, out_d):
    f32 = mybir.dt.float32
    f16 = mybir.dt.float16
    bf16 = mybir.dt.bfloat16
    f8 = mybir.dt.float8e4
    DR = mybir.MatmulPerfMode.DoubleRow

    expp, rpool, outp, psq_pool, pssc_pool, psctx_pool = pools
    xt8_sb = tiles["xt8"]
    wqk8_sb = tiles["wqk8"]
    xtb_sb = tiles["xtb"]
    wv_sb = tiles["wv"]
    wo_sb = tiles["wo"]
    qt_sb = tiles["qt"]
    kt_sb = tiles["kt"]
    v_sb = tiles["v"]
    ctxt_sb = tiles["ctxt"]
    tri_sb = tiles["tri"]
    if True:
        def pair3(tile, n_inner):  # [128, 2*n] -> [128, 2, n]
            return tile.rearrange("p (s n) -> p s n", s=2)

        # ---- input DMA, split across queues ----
        for i in range(NPAIR):
            nc.sync.dma_start(out=xt8_sb[i][:], in_=xt8_d[i * 128 : (i + 1) * 128, :])
            nc.sync.dma_start(
                out=wqk8_sb[i][:], in_=wqk8_d[i * 128 : (i + 1) * 128, :]
            )
        for k in range(KT):
            nc.gpsimd.dma_start(
                out=xtb_sb[k][:], in_=xtb_d[k * 128 : (k + 1) * 128, :]
            )
            nc.gpsimd.dma_start(out=wv_sb[k][:], in_=wv_d[k * 128 : (k + 1) * 128, :])
        for p in range(4):
            nc.gpsimd.dma_start(out=wo_sb[p][:], in_=wo_d[p * 128 : (p + 1) * 128, :])

        # ---- work-stream closures ----
        def qk_proj_group(qc, which, hp):
            # one PSUM tile: 128 cols = heads (2hp, 2hp+1) x 64 hd-dims
            def emit():
                nsl = slice(qc * 512, (qc + 1) * 512)
                ps = psq_pool.tile(
                    [128, 512], f32, name=f"pq{qc}{which}{hp}{sfx}", tag="psq"
                )
                base = which * 512 + hp * 128
                for pair in range(NPAIR):
                    nc.tensor.matmul(
                        ps[:],
                        lhsT=pair3(wqk8_sb[pair], 1024)[:, :, base : base + 128],
                        rhs=pair3(xt8_sb[pair], N_SEQ)[:, :, nsl],
                        start=(pair == 0),
                        stop=(pair == NPAIR - 1),
                        perf_mode=DR,
                    )
                dst = qt_sb[hp] if which == 0 else kt_sb[hp]
                nc.vector.tensor_copy(dst[:, nsl], ps[:])

            return emit

        def v_proj_group(nt):
            def emit():
                ps = psq_pool.tile([128, 512], f32, name=f"pv{nt}{sfx}", tag="psq")
                for k in range(KT):
                    nc.tensor.matmul(
                        ps[:],
                        lhsT=xtb_sb[k][:, nt * 128 : (nt + 1) * 128],
                        rhs=wv_sb[k][:],
                        start=(k == 0),
                        stop=(k == KT - 1),
                    )
                v_view = v_sb[nt].rearrange("p (h c) -> p h c", h=8)
                nc.vector.tensor_copy(
                    v_view[:, :, 0:64], ps.rearrange("p (h c) -> p h c", h=8)
                )

            return emit

        def outproj_group(nt):
            def emit():
                osb = outp.tile([128, D_EMB], f16, name=f"osb{nt}{sfx}", tag="osb")
                for dh in range(2):
                    pso = psq_pool.tile(
                        [128, 512], f32, name=f"po{nt}{dh}{sfx}", tag="psq"
                    )
                    for hp in range(4):
                        nc.tensor.matmul(
                            pso[:],
                            lhsT=ctxt_sb[hp][:, nt * 128 : (nt + 1) * 128],
                            rhs=wo_sb[hp][:, dh * 512 : (dh + 1) * 512],
                            start=(hp == 0),
                            stop=(hp == 3),
                        )
                    nc.vector.tensor_copy(osb[:, dh * 512 : (dh + 1) * 512], pso[:])
                nc.sync.dma_start(
                    out=out_d[nt * 128 : (nt + 1) * 128, :], in_=osb[:]
                )

            return emit

        def proj_chunk_groups(qc):
            gs = [qk_proj_group(qc, w, hp) for w in (0, 1) for hp in range(4)]
            gs += [v_proj_group(nt) for nt in range(4 * qc, 4 * qc + 4)]
            return gs

        # ---- attention ----
        def emit_scores(qc, hp, ki, ps_tiles):
            # ps [128, 1024]: k-tile ki x (2 heads x 512 q); fp8 DoubleRow
            jj = ki - 4 * qc
            t0 = max(0, 128 * jj)
            q0 = qc * 512
            ps = pssc_pool.tile(
                [128, 1024], f32, name=f"sc{qc}_{hp}_{ki}{sfx}", tag="sc"
            )
            ps_tiles[(hp, ki)] = ps
            for h2 in range(2):
                hb = h2 * 64
                nc.tensor.matmul(
                    ps[:, h2 * 512 + t0 : (h2 + 1) * 512],
                    lhsT=kt_sb[hp][hb : hb + 64, ki * 128 : (ki + 1) * 128],
                    rhs=qt_sb[hp][hb : hb + 64, q0 + t0 : q0 + 512],
                    start=True,
                    stop=True,
                )

        for qc in range(QC):
            q0 = qc * 512
            nk = 4 * qc + 4

            if qc == 0:
                for g in proj_chunk_groups(0):
                    g()

            F = []
            if qc + 1 < QC:
                F += proj_chunk_groups(qc + 1)
            if qc >= 1:
                F += [outproj_group(nt) for nt in range(4 * (qc - 1), 4 * qc)]

            groups = [(hp, ki) for hp in range(4) for ki in range(nk)]
            steps = len(groups)
            inc = len(F) / steps
            acc = 0.0
            fi = 0
            ps_tiles = {}
            ctx_ps = {}

            emit_scores(qc, groups[0][0], groups[0][1], ps_tiles)
            for idx, (hp, ki) in enumerate(groups):
                jj = ki - 4 * qc
                t0 = max(0, 128 * jj)
                if ki == 0:
                    for h2 in range(2):
                        ctx_ps[(hp, h2)] = psctx_pool.tile(
                            [128, 512], f32, name=f"ctx{qc}_{hp}_{h2}{sfx}", tag="ctx"
                        )
                if idx + 1 < steps:
                    emit_scores(qc, groups[idx + 1][0], groups[idx + 1][1], ps_tiles)

                ps = ps_tiles.pop((hp, ki))
                ex = expp.tile(
                    [128, 1024], bf16, name=f"ex{qc}_{hp}_{ki}{sfx}", tag="ex"
                )
                if t0:
                    nc.scalar.activation(
                        ex.rearrange("p (h n) -> p h n", h=2)[:, :, t0:512],
                        ps.rearrange("p (h n) -> p h n", h=2)[:, :, t0:512],
                        mybir.ActivationFunctionType.Exp,
                        scale=float(SCALE_EXP),
                    )
                else:
                    nc.scalar.activation(
                        ex[:],
                        ps[:],
                        mybir.ActivationFunctionType.Exp,
                        scale=float(SCALE_EXP),
                    )
                if jj >= 0:  # triangular mask on the diagonal 128-block
                    for h2 in range(2):
                        blk = slice(h2 * 512 + t0, h2 * 512 + t0 + 128)
                        nc.vector.tensor_mul(ex[:, blk], ex[:, blk], tri_sb[:])

                acc += inc
                while acc >= 1.0 and fi < len(F):
                    F[fi]()
                    fi += 1
                    acc -= 1.0

                for h2 in range(2):
                    h = 2 * hp + h2
                    nc.tensor.matmul(
                        ctx_ps[(hp, h2)][:, t0:512],
                        lhsT=v_sb[ki][:, h * 128 : (h + 1) * 128],
                        rhs=ex[:, h2 * 512 + t0 : (h2 + 1) * 512],
                        start=(ki == 0),
                        stop=(ki == nk - 1),
                    )

                if ki == nk - 1:  # normalize this head-pair
                    for h2 in range(2):
                        cp = ctx_ps.pop((hp, h2))
                        rec = rpool.tile(
                            [64, 512], f32, name=f"rec{qc}_{hp}_{h2}{sfx}", tag="rec"
                        )
                        nc.vector.reciprocal(rec[:], cp[64:128, :])
                        nc.vector.tensor_mul(
                            ctxt_sb[hp][h2 * 64 : h2 * 64 + 64, q0 : q0 + 512],
                            cp[0:64, :],
                            rec[:],
                        )

            while fi < len(F):
                F[fi]()
                fi += 1

        # last chunk's out-proj has no later chunk to hide in
        for nt in range(4 * (QC - 1), 4 * QC):
            outproj_group(nt)()


def _build_module(reps=1):
    import concourse.bacc as bacc
    import concourse.mybir as mybir
    import concourse.tile as tile

    f16 = mybir.dt.float16
    f32 = mybir.dt.float32
    bf16 = mybir.dt.bfloat16
    f8 = mybir.dt.float8e4

    nc = bacc.Bacc()
    xt8_d = nc.dram_tensor("xt8", [512, 2 * N_SEQ], f8, kind="ExternalInput")
    wqk8_d = nc.dram_tensor("wqk8", [512, 2048], f8, kind="ExternalInput")
    xtb_d = nc.dram_tensor("xtb", [D_EMB, N_SEQ], bf16, kind="ExternalInput")
    wv_d = nc.dram_tensor("wv", [D_EMB, 512], bf16, kind="ExternalInput")
    wo_d = nc.dram_tensor("wo", [512, D_EMB], bf16, kind="ExternalInput")
    out_d = nc.dram_tensor("out", [N_SEQ, D_EMB], f16, kind="ExternalOutput")

    with tile.TileContext(nc) as tc:
        with (
            tc.tile_pool(name="persist", bufs=1) as persist,
            tc.tile_pool(name="expp", bufs=4) as expp,
            tc.tile_pool(name="rpool", bufs=4) as rpool,
            tc.tile_pool(name="outp", bufs=3) as outp,
            tc.tile_pool(name="psq", bufs=2, space="PSUM") as psq_pool,
            tc.tile_pool(name="pssc", bufs=2, space="PSUM") as pssc_pool,
            tc.tile_pool(name="psctx", bufs=2, space="PSUM") as psctx_pool,
        ):
            pools = (expp, rpool, outp, psq_pool, pssc_pool, psctx_pool)
            tiles = {
                "xt8": [persist.tile([128, 2 * N_SEQ], f8, name=f"xt8_{i}") for i in range(NPAIR)],
                "wqk8": [persist.tile([128, 2048], f8, name=f"wqk8_{i}") for i in range(NPAIR)],
                "xtb": [persist.tile([128, N_SEQ], bf16, name=f"xtb{k}") for k in range(KT)],
                "wv": [persist.tile([128, 512], bf16, name=f"wv{k}") for k in range(KT)],
                "wo": [persist.tile([128, D_EMB], bf16, name=f"wo{p}") for p in range(4)],
                "qt": [persist.tile([128, N_SEQ], bf16, name=f"qt{hp}") for hp in range(4)],
                "kt": [persist.tile([128, N_SEQ], bf16, name=f"kt{hp}") for hp in range(4)],
                # v per n-tile [128, 1024]: head h -> cols h*128:h*128+64 = v_h,
                # cols h*128+64:h*128+128 = 1.0 (softmax denominator ones-trick)
                "v": [persist.tile([128, 1024], bf16, name=f"v{nt}") for nt in range(NT)],
                "ctxt": [persist.tile([128, N_SEQ], bf16, name=f"ctxt{p}") for p in range(4)],
                "tri": persist.tile([128, 128], bf16, name="tri"),
            }
            # constants (ones blocks + causal triangle), once for all reps
            for nt in range(NT):
                ones_view = tiles["v"][nt].rearrange("p (h c) -> p h c", h=8)
                nc.gpsimd.memset(ones_view[:, :, 64:128], 1.0)
            # tri[k_local, q_local] = 1.0 if q_local >= k_local else 0
            nc.gpsimd.memset(tiles["tri"][:], 1.0)
            nc.gpsimd.affine_select(
                out=tiles["tri"][:],
                in_=tiles["tri"][:],
                compare_op=mybir.AluOpType.is_ge,
                fill=0.0,
                base=0,
                pattern=[[1, 128]],
                channel_multiplier=-1,
            )
            for rep in range(reps):
                _emit_body(
                    nc, tc, mybir, f"_r{rep}" if reps > 1 else "", rep, pools, tiles,
                    xt8_d, wqk8_d, xtb_d, wv_d, wo_d, out_d,
                )

    if not nc.is_finalized():
        nc.finalize()
    return nc


def _get_module(reps=1):
    key = f"nc{reps}"
    if key not in _CACHE:
        _CACHE[key] = _build_module(reps)
    return _CACHE[key]


def _pairs(a, ncols):
    # [1024, ncols] -> [512, 2*ncols]: row pair*128+p, col sub*ncols+c
    return (
        a.reshape(NPAIR, 2, 128, ncols).transpose(0, 2, 1, 3).reshape(512, 2 * ncols)
    )


def _f8(a):
    return np.clip(a, -240.0, 240.0).astype(F8E4)


def make_in_maps(x, W_q, W_k, W_v, W_o):
    x = np.asarray(x, np.float32)
    in_maps = []
    for c in range(8):
        b, g = c // 2, c % 2
        gs = slice(g * 512, (g + 1) * 512)
        xT = np.ascontiguousarray(x[b].T)  # [1024, 2048]
        wq = np.asarray(W_q[:, gs], np.float32) * WS
        wk = np.asarray(W_k[:, gs], np.float32) * WS
        wqk = np.concatenate([wq, wk], axis=1)  # [1024, 1024]
        in_maps.append(
            {
                "xt8": _f8(_pairs(xT, N_SEQ)),
                "wqk8": _f8(_pairs(wqk, 1024)),
                "xtb": xT.astype(BF16),
                "wv": np.ascontiguousarray(W_v[:, gs]).astype(BF16),
                "wo": np.ascontiguousarray(W_o[gs, :]).astype(BF16),
            }
        )
    return in_maps


def kernel(x, W_q, W_k, W_v, W_o, b_o):
    from concourse.bass_utils import run_bass_kernel_spmd

    nc = _get_module()
    in_maps = make_in_maps(x, W_q, W_k, W_v, W_o)
    res = run_bass_kernel_spmd(nc, in_maps, core_ids=list(range(8)))

    out = np.empty((4, N_SEQ, D_EMB), np.float32)
    for b in range(4):
        out[b] = (
            res.results[2 * b]["out"].astype(np.float32)
            + res.results[2 * b + 1]["out"].astype(np.float32)
            + np.asarray(b_o, np.float32)[None, :]
        )
    return out
